# revision 1
# baseline (speedup 1.0000x reference)
"""Trainium2 Bass kernel for nn_ClusterlingLayer (ragged_sequence).

Computes, for B=131072 fibers against K=64 clusters:
  x_dis[b,k] = ||x_b||^2 + ||w_k||^2 - 2 x_b.w_k
  dice[b,k]  = 1 - (2*inter + s)/(nF + nC + s)   (inter = ragged ROI histogram dot)
  q = rownorm( 1 / (1 + x_dis*dice) )
Returns (q, x_dis) like the reference.

Sharding: data-parallel over B across 8 NeuronCores (16384 fibers/core).
Host prep is limited to layout transforms (x transpose, dtype casts), input
norms, and K-side constants (cluster histogram table from cluster_rois -
4k elements). All B-proportional compute (matmul, per-fiber ROI histograms,
dice/t-distribution math) runs on device.

Device strategy per 128-fiber subtile:
 - per-fiber vocab histogram on DVE via two custom fused DVE ops registered
   at build time (HIST3F: 3 is_equal compares -> fresh partial; HIST2:
   2 compares + accumulate), 12 DVE instructions per subtile in two
   independent chains (vs 24 unfused ops). Masked rois point at a
   sentinel bin (128) so no separate mask multiply is needed.
 - PE transposes the bf16 histogram -> [vocab, fiber] (identity matmul),
   ACT copies it back to SBUF, PE contracts with the (-2x) cluster table
   so PSUM holds -2*inter directly.
 - x_dis via PE: 4 accumulating (-2 w^T) d-chunks + a rank-2 augment row
   pair that folds in ||x||^2 (host-computed) and ||w||^2.
 - elementwise uses the single-reciprocal form
   q_un = den/(den + x_dis*(den0 - 2*inter)), den = den0 + s, den0 = nF+nC
   with the generic formula covering the empty-set special cases to within
   ~1e-9 of the reference. Plain tensor_tensor ops run on GPSIMD, the
   reciprocal and row-reduction on DVE, PSUM->SBUF copies on ACT; big DMAs
   are issued from SP/ACT HWDGE queues (Pool SWDGE descriptor generation
   would serialize with GPSIMD compute).
"""

import os
import sys

import numpy as np

for _p in ("/opt/trn_rl_repo", os.path.expanduser("~/.axon_site/_ro/trn_rl_repo")):
    if os.path.isdir(_p) and _p not in sys.path:
        sys.path.insert(0, _p)

import concourse.bass as bass
import concourse.mybir as mybir
import concourse.tile as tile
from concourse import bacc
from concourse.bass_utils import run_bass_kernel_spmd

import ml_dtypes


def _register_hist_ops():
    """Register fused histogram DVE ops (2-3 is_equal compares + accumulate
    per instruction) in the custom-DVE registry. Self-pins the uop shas."""
    from concourse import dve_ops
    from concourse.dve_spec import (
        Spec, Src0, Src1, C0, C1, C3, eq, _spill_c3_to_src1, lower,
        _has_src1 as has_src1,
    )

    if "HIST2_ANT" in dve_ops._SUB_OPCODE_FOR_NAME:
        return

    h2 = dve_ops.DveOp(
        "HIST2_ANT",
        Spec(
            body=eq(Src0, C0) + eq(Src0, C1) + Src1,
            reference=lambda in0, in1, s0, s1, imm2: (
                (in0 == s0) + (in0 == s1) + in1
            ).astype(np.float32),
        ),
        subdim=False,
        uops_sha={},
    )
    h3 = dve_ops.DveOp(
        "HIST3F_ANT",
        Spec(
            body=_spill_c3_to_src1(eq(Src0, C0) + eq(Src0, C1) + eq(Src0, C3)),
            reference=lambda in0, in1, s0, s1, imm2: (
                (in0 == s0) + (in0 == s1) + (in0 == in1.reshape(-1, 1)[:, :1])
            ).astype(np.float32),
        ),
        subdim=False,
        uops_sha={},
    )
    for op in (h2, h3):
        dve_ops.OPS.append(op)
        dve_ops.CUSTOM_DVE_SPECS[op.name] = op.spec
        dve_ops._SUB_OPCODE_FOR_NAME[op.name] = (
            max(dve_ops._SUB_OPCODE_FOR_NAME.values()) + 1
        )
    for op in (h2, h3):
        for ver in ("v3", "v4"):
            spec_c = dve_ops.DveOpSpec(
                name=op.name,
                opcode=dve_ops.get_dve_sub_opcode(op.name),
                uops=lower(op.spec, ver=ver),
                rd1_en=has_src1(op.spec),
            )
            op.uops_sha[ver] = spec_c.sha(ver)
    return

NCORES = 8
B, D, K, LF, LC = 131072, 512, 64, 24, 64
V = 128            # ROI vocab
BS = B // NCORES   # fibers per core
SUB = 128          # fibers per subtile (partition dim)
GRAN = 512         # fibers per granule
NGRAN = BS // GRAN
NSUB = GRAN // SUB
SMOOTH = 1e-6
HB = 130           # histogram bins incl. sentinel 128 (+pad to even)

f32 = mybir.dt.float32
bf16 = mybir.dt.bfloat16
i16 = mybir.dt.int16


def _build_nc(skip_hist=False, skip_mm=False, skip_ew=False, skip_xdma=False,
              gsplit=1000000):
    _register_hist_ops()
    nc = bacc.Bacc("TRN2", target_bir_lowering=False)

    xT = nc.dram_tensor("xT", [D, BS], f32, kind="ExternalInput")
    aug = nc.dram_tensor("aug", [2, BS], f32, kind="ExternalInput")
    rl = nc.dram_tensor("rl", [BS, LF + 1], i16, kind="ExternalInput")
    wT2 = nc.dram_tensor("wT2", [D, K], f32, kind="ExternalInput")
    wsq1 = nc.dram_tensor("wsq1", [2, K], f32, kind="ExternalInput")
    tbl = nc.dram_tensor("tbl", [V, K], bf16, kind="ExternalInput")
    ncs = nc.dram_tensor("ncs", [NSUB * K], f32, kind="ExternalInput")
    iotav = nc.dram_tensor("iotav", [HB], bf16, kind="ExternalInput")
    iotaj = nc.dram_tensor("iotaj", [LF], bf16, kind="ExternalInput")
    ident = nc.dram_tensor("ident", [SUB, SUB], bf16, kind="ExternalInput")

    q_out = nc.dram_tensor("q_out", [BS, K], f32, kind="ExternalOutput")
    xd_out = nc.dram_tensor("xd_out", [BS, K], f32, kind="ExternalOutput")

    # handy rearranged views of DRAM tensors
    xT_v = xT[:].rearrange("(c p) n -> p c n", p=SUB)          # [128, 4, BS]
    rl_v = rl[:].rearrange("(t p) j -> p t j", p=SUB)          # [128, BS/128, 25]
    def out_view(t, g):
        # [128 p, NSUB t, 64 k] slice of a [BS, K] output for granule g
        return bass.AP(tensor=t, offset=g * GRAN * K,
                       ap=[[K, SUB], [SUB * K, NSUB], [1, K]])

    def bcast_row(dram_ap, n):
        # DMA-read AP replicating a DRAM row across n partitions
        return bass.AP(
            tensor=dram_ap.tensor,
            offset=dram_ap.offset,
            ap=[[0, n]] + dram_ap.ap,
        )

    with tile.TileContext(nc) as tc:
        with (
            tc.tile_pool(name="consts", bufs=1) as consts,
            tc.tile_pool(name="xin", bufs=4) as xin,
            tc.tile_pool(name="rin", bufs=6) as rin,
            tc.tile_pool(name="prep", bufs=6) as prep,
            tc.tile_pool(name="hist", bufs=12) as hist,
            tc.tile_pool(name="histT", bufs=8) as histT,
            tc.tile_pool(name="ew", bufs=6) as ew,
            tc.tile_pool(name="outs", bufs=6) as outs,
            tc.tile_pool(name="psx", bufs=3, space="PSUM") as psx,
            tc.tile_pool(name="psi", bufs=3, space="PSUM") as psi,
            tc.tile_pool(name="pst", bufs=2, space="PSUM") as pst,
        ):
            # ---- constants (loaded once) ----
            c_wT = consts.tile([SUB, 4, K], f32)
            nc.sync.dma_start(out=c_wT, in_=wT2[:].rearrange("(c p) k -> p c k", p=SUB))
            c_wsq1 = consts.tile([2, K], f32)
            nc.sync.dma_start(out=c_wsq1, in_=wsq1[:])
            c_tbl = consts.tile([V, K], bf16)
            nc.sync.dma_start(out=c_tbl, in_=tbl[:])
            c_ncs = consts.tile([SUB, NSUB * K], f32)
            nc.sync.dma_start(out=c_ncs, in_=bcast_row(ncs[:], SUB))
            c_iov = consts.tile([SUB, HB], bf16)
            nc.sync.dma_start(out=c_iov, in_=bcast_row(iotav[:], SUB))
            c_ioj = consts.tile([SUB, LF], bf16)
            nc.sync.dma_start(out=c_ioj, in_=bcast_row(iotaj[:], SUB))
            c_id = consts.tile([SUB, SUB], bf16)
            nc.sync.dma_start(out=c_id, in_=ident[:])
            c_zero = consts.tile([SUB, HB], bf16)
            nc.vector.memset(c_zero, 0.0)
            c_srep = consts.tile([SUB, NSUB * K], f32)
            nc.gpsimd.memset(c_srep, SMOOTH)
            c_aug = consts.tile([2, BS], f32)
            nc.sync.dma_start(out=c_aug, in_=aug[:])

            for g in range(NGRAN):
                t0 = g * NSUB  # first subtile index within rearranged views

                xt = xin.tile([SUB, 4, GRAN], f32, tag="xt")
                if not skip_xdma:
                    nc.sync.dma_start(out=xt, in_=xT_v[:, :, g * GRAN:(g + 1) * GRAN])
                else:
                    nc.sync.dma_start(out=xt[:, :, 0:1],
                                      in_=xT_v[:, :, g * GRAN:g * GRAN + 1])
                rt = rin.tile([SUB, NSUB, LF + 1], i16, tag="rt")
                nc.scalar.dma_start(out=rt, in_=rl_v[:, t0:t0 + NSUB, :])

                # ---- roi prep: int16 -> bf16, mask invalid j to sentinel 128
                rb = prep.tile([SUB, NSUB, LF], f32, tag="rb")
                nc.vector.tensor_copy(out=rb, in_=rt[:, :, 0:LF])
                lt = prep.tile([SUB, NSUB], f32, tag="lt")
                nc.vector.tensor_copy(out=lt, in_=rt[:, :, LF])
                mk = prep.tile([SUB, NSUB, LF], f32, tag="mk")
                for s in range(NSUB):
                    nc.vector.tensor_scalar(
                        out=mk[:, s, :], in0=c_ioj, scalar1=lt[:, s:s + 1],
                        scalar2=None, op0=mybir.AluOpType.is_lt,
                    )
                mr = prep.tile([SUB, NSUB, LF], f32, tag="mr")
                # mr = (rb - 128)*mask + 128  -> roi for valid j, 128 sentinel else
                nc.vector.scalar_tensor_tensor(
                    out=mr, in0=rb, scalar=-128.0, in1=mk,
                    op0=mybir.AluOpType.add, op1=mybir.AluOpType.mult,
                )
                nc.vector.tensor_scalar(
                    out=mr, in0=mr, scalar1=128.0, scalar2=None,
                    op0=mybir.AluOpType.add,
                )

                psum_x = psx.tile([SUB, NSUB * K], f32, tag="px")
                psum_i = psi.tile([SUB, NSUB * K], f32, tag="pi")

                for s in range(NSUB):
                    # ---- per-fiber vocab histogram.
                    # DVE subtiles: 11 fused custom ops (HIST3F starts a chain
                    # with 3 compares, HIST2 adds 2 compares each) in two
                    # independent chains + one merge add.
                    # Every gsplit-th subtile runs on GPSIMD instead (plain
                    # fused compare+add STT ops) to share the load.
                    from concourse.dve_ops import OPS as _OPS
                    _h2 = next(o for o in _OPS if o.name == "HIST2_ANT")
                    _h3 = next(o for o in _OPS if o.name == "HIST3F_ANT")
                    sc = lambda j: mr[:, s, j:j + 1]
                    if not skip_hist:
                        on_gpsimd = ((g * NSUB + s) % gsplit) == (gsplit - 1)
                        if on_gpsimd:
                            ha = hist.tile([SUB, HB], bf16, tag="ga")
                            hc = hist.tile([SUB, HB], bf16, tag="gb")
                            nc.gpsimd.tensor_scalar(
                                out=ha, in0=c_iov, scalar1=sc(0),
                                scalar2=None, op0=mybir.AluOpType.is_equal,
                            )
                            cur, nxt = ha, hc
                            for j in range(1, LF):
                                nc.gpsimd.scalar_tensor_tensor(
                                    out=nxt, in0=c_iov, scalar=sc(j), in1=cur,
                                    op0=mybir.AluOpType.is_equal,
                                    op1=mybir.AluOpType.add,
                                )
                                cur, nxt = nxt, cur
                        else:
                            a0 = hist.tile([SUB, HB], bf16, tag="a0")
                            a1 = hist.tile([SUB, HB], bf16, tag="a1")
                            b0 = hist.tile([SUB, HB], bf16, tag="b0")
                            b1 = hist.tile([SUB, HB], bf16, tag="b1")
                            nc.vector._custom_dve(
                                _h3, out=a0, in0=c_iov, in1=sc(2),
                                s0=sc(0), s1=sc(1))
                            nc.vector._custom_dve(
                                _h3, out=b0, in0=c_iov, in1=sc(13),
                                s0=sc(11), s1=sc(12))
                            ca, na = a0, a1
                            for j0 in (3, 5, 7, 9):
                                nc.vector._custom_dve(
                                    _h2, out=na, in0=c_iov, in1=ca,
                                    s0=sc(j0), s1=sc(j0 + 1))
                                ca, na = na, ca
                            cb, nb = b0, b1
                            for j0 in (14, 16, 18, 20):
                                nc.vector._custom_dve(
                                    _h2, out=nb, in0=c_iov, in1=cb,
                                    s0=sc(j0), s1=sc(j0 + 1))
                                cb, nb = nb, cb
                            nc.vector._custom_dve(
                                _h2, out=na, in0=c_iov, in1=ca,
                                s0=sc(22), s1=sc(23))
                            nc.vector.tensor_tensor(
                                out=na, in0=na, in1=cb, op=mybir.AluOpType.add)
                            cur = na
                    else:
                        cur = c_zero
                    pt = pst.tile([SUB, SUB], bf16, tag="pt")
                    nc.tensor.transpose(out=pt, in_=cur[:, 0:V], identity=c_id)
                    hT = histT.tile([V, SUB], bf16, tag="hT")
                    nc.scalar.copy(out=hT, in_=pt)
                    # inter[p, k] for this subtile
                    nc.tensor.matmul(
                        psum_i[:, s * K:(s + 1) * K], lhsT=hT, rhs=c_tbl,
                        start=True, stop=True,
                    )
                    # x_dis[p, k]: 4 chunks of (-2 w^T) + rank-2 augment
                    if not skip_mm:
                        for c in range(4):
                            nc.tensor.matmul(
                                psum_x[:, s * K:(s + 1) * K],
                                lhsT=xt[:, c, s * SUB:(s + 1) * SUB],
                                rhs=c_wT[:, c, :],
                                start=(c == 0), stop=False,
                            )
                        nc.tensor.matmul(
                            psum_x[:, s * K:(s + 1) * K],
                            lhsT=c_aug[:, g * GRAN + s * SUB:g * GRAN + (s + 1) * SUB],
                            rhs=c_wsq1,
                            start=False, stop=True,
                        )
                    else:
                        nc.tensor.matmul(
                            psum_x[:, s * K:(s + 1) * K],
                            lhsT=c_aug[:, g * GRAN + s * SUB:g * GRAN + (s + 1) * SUB],
                            rhs=c_wsq1,
                            start=True, stop=True,
                        )

                # ---- elementwise on the full granule [128, 256] ----
                xd = outs.tile([SUB, NSUB * K], f32, tag="xd")
                nc.scalar.copy(out=xd, in_=psum_x)  # ACT: PSUM -> SBUF

                if skip_ew:
                    nc.sync.dma_start(out=out_view(q_out, g), in_=xd[:])
                    nc.sync.dma_start(out=out_view(xd_out, g), in_=xd[:])
                    continue
                isb = ew.tile([SUB, NSUB * K], f32, tag="isb")
                nc.scalar.copy(out=isb, in_=psum_i)  # ACT: -2*inter, PSUM->SBUF
                den0 = ew.tile([SUB, NSUB * K], f32, tag="den0")
                lt_ap = lt[:]
                lt_b = bass.AP(
                    tensor=lt_ap.tensor, offset=lt_ap.offset,
                    ap=list(lt_ap.ap) + [[0, K]],
                )
                nc.gpsimd.tensor_tensor(
                    out=den0, in0=lt_b, in1=c_ncs, op=mybir.AluOpType.add,
                )
                dens = ew.tile([SUB, NSUB * K], f32, tag="dens")
                nc.gpsimd.tensor_tensor(
                    out=dens, in0=den0, in1=c_srep, op=mybir.AluOpType.add,
                )
                a = ew.tile([SUB, NSUB * K], f32, tag="a")
                nc.gpsimd.tensor_tensor(
                    out=a, in0=isb, in1=den0, op=mybir.AluOpType.add,
                )
                b = ew.tile([SUB, NSUB * K], f32, tag="b")
                nc.gpsimd.tensor_tensor(
                    out=b, in0=a, in1=xd, op=mybir.AluOpType.mult,
                )
                cden = ew.tile([SUB, NSUB * K], f32, tag="cden")
                nc.gpsimd.tensor_tensor(
                    out=cden, in0=b, in1=dens, op=mybir.AluOpType.add,
                )
                rc = ew.tile([SUB, NSUB * K], f32, tag="rc")
                nc.vector.reciprocal(out=rc, in_=cden)
                qn = ew.tile([SUB, NSUB * K], f32, tag="qn")
                nc.gpsimd.tensor_tensor(
                    out=qn, in0=dens, in1=rc, op=mybir.AluOpType.mult,
                )
                rs = ew.tile([SUB, NSUB], f32, tag="rs")
                nc.vector.tensor_reduce(
                    out=rs, in_=qn[:].rearrange("p (t k) -> p t k", k=K),
                    axis=mybir.AxisListType.X, op=mybir.AluOpType.add,
                )
                rn = ew.tile([SUB, NSUB], f32, tag="rn")
                nc.vector.reciprocal(out=rn, in_=rs)
                qf = outs.tile([SUB, NSUB * K], f32, tag="qf")
                rn_ap = rn[:]
                rn_b = bass.AP(
                    tensor=rn_ap.tensor, offset=rn_ap.offset,
                    ap=list(rn_ap.ap) + [[0, K]],
                )
                nc.gpsimd.tensor_tensor(
                    out=qf, in0=qn[:].rearrange("p (t k) -> p t k", k=K),
                    in1=rn_b, op=mybir.AluOpType.mult,
                )

                nc.sync.dma_start(out=out_view(q_out, g), in_=qf[:])
                nc.sync.dma_start(out=out_view(xd_out, g), in_=xd[:])

    nc.finalize()  # runs Bacc.compile(): wait-splitting, reg alloc, nop fusion
    return nc


_NC_CACHE = None
_LAST = None


def _get_nc():
    global _NC_CACHE
    if _NC_CACHE is None:
        _NC_CACHE = _build_nc()
    return _NC_CACHE


def kernel(x, weight, fiber_rois, fiber_lens, cluster_rois, cluster_lens):
    x = np.asarray(x, np.float32)
    weight = np.asarray(weight, np.float32)
    fiber_rois = np.asarray(fiber_rois, np.int32)
    fiber_lens = np.asarray(fiber_lens, np.int32)
    cluster_rois = np.asarray(cluster_rois, np.int32)
    cluster_lens = np.asarray(cluster_lens, np.int32)

    # K-side host prep (tiny): cluster histogram table, norms, constants
    mC = (np.arange(LC)[None, :] < cluster_lens[:, None])
    histC = np.zeros((K, V), np.float32)
    for k in range(K):
        histC[k] = np.bincount(cluster_rois[k][mC[k]], minlength=V).astype(np.float32)
    tbl = (-2.0 * histC.T).astype(ml_dtypes.bfloat16)      # [V, K], -2 folded in
    wsq = (weight * weight).sum(1).astype(np.float32)       # [K]
    nC = cluster_lens.astype(np.float32)
    wsq1 = np.stack([wsq, np.ones(K, np.float32)])          # [2, K]
    ncs = np.tile(nC, NSUB).astype(np.float32)              # [256]
    iotav = np.arange(HB).astype(ml_dtypes.bfloat16)
    iotaj = np.arange(LF).astype(ml_dtypes.bfloat16)
    ident = np.eye(SUB).astype(ml_dtypes.bfloat16)

    xsq = np.einsum("bd,bd->b", x, x).astype(np.float32)    # input norms
    wT2 = (-2.0 * weight.T).astype(np.float32)              # [D, K]

    nc = _get_nc()
    in_maps = []
    for ci in range(NCORES):
        sl = slice(ci * BS, (ci + 1) * BS)
        in_maps.append({
            "xT": np.ascontiguousarray(x[sl].T),
            "aug": np.ascontiguousarray(
                np.stack([np.ones(BS, np.float32), xsq[sl]])),
            "rl": np.ascontiguousarray(np.concatenate(
                [fiber_rois[sl], fiber_lens[sl][:, None]], axis=1).astype(np.int16)),
            "wT2": wT2,
            "wsq1": wsq1,
            "tbl": tbl,
            "ncs": ncs,
            "iotav": iotav,
            "iotaj": iotaj,
            "ident": ident,
        })

    res = run_bass_kernel_spmd(nc, in_maps, core_ids=list(range(NCORES)))
    global _LAST
    _LAST = res
    q = np.concatenate([r["q_out"] for r in res.results], axis=0)
    xd = np.concatenate([r["xd_out"] for r in res.results], axis=0)
    return (q, xd)

